# revision 4
# baseline (speedup 1.0000x reference)
"""Trainium2 Bass kernel for nn_NodeBlock (GNN message passing).

Math (reference):
    c1  = concat([node_emb[i], edge_emb], axis=1)          # [E, 128]
    y   = LayerNorm(c1 @ W.T + b) * g1 + b1                # [E, 128]
    msg = sigmoid(y[:, :64]) * tanh(y[:, 64:])             # [E, 64]
    agg = segment_sum(msg, i, N)                           # [N, 64]
    out = tanh(node_emb + LayerNorm(agg) * g2 + b2)        # [N, 64]

Strategy: host-side sort edges by destination node; group nodes into
blocks of 128 and pad each block's edge list to a uniform multiple of
128; assign 98 consecutive blocks to each of the 8 cores (edges of a
block never cross cores, so no all-reduce is needed). On device, per
128-edge subtile: build one-hot matrices from the local node index, use
them both to expand the per-block node projection (node_emb @ Wn.T + b,
precomputed per block on device) into per-edge rows via matmul, and to
segment-sum msg into per-block [128, 64] PSUM accumulators. LayerNorm
runs row-wise (edges on partitions, features on the free dim).
"""

import os
import sys
import numpy as np

for _p in ("/root/.axon_site", "/root/.axon_site/_ro/trn_rl_repo",
           "/root/.axon_site/_ro/pypackages", "/opt/trn_rl_repo"):
    if os.path.isdir(_p) and _p not in sys.path:
        sys.path.append(_p)

NCORES = 8
NB = 128           # nodes per block
D = 64             # node/edge feature dim
DO = 128           # linear output dim (2*D)
SUPER = 8          # subtiles (128 edges each) processed per super-tile
EPS = 1e-5

_NC_CACHE = {}


def _build_nc(nsub, nsup, bpc, ln1_triv, ln2_triv):
    key = (nsub, nsup, bpc, ln1_triv, ln2_triv)
    if key in _NC_CACHE:
        return _NC_CACHE[key]

    import concourse.bacc as bacc
    import concourse.mybir as mybir
    from concourse.tile import TileContext

    F32 = mybir.dt.float32
    AF = mybir.ActivationFunctionType
    OP = mybir.AluOpType
    AX = mybir.AxisListType

    S = bpc * nsub                    # real subtiles per core
    SW = SUPER * NB                   # edges per super-tile
    NPCOL = bpc * NB                  # node_proj columns
    ngrp = (bpc + 7) // 8             # phase-C groups of 8 blocks

    # subtile -> local block id (pad subtiles at the end belong to the last
    # block; their one-hot columns are all-zero so they contribute nothing)
    bs = [min(s // nsub, bpc - 1) for s in range(nsup * SUPER)]

    nc = bacc.Bacc("TRN2", target_bir_lowering=False)

    edgeT_d = nc.dram_tensor("edgeT", [nsup, D, SW], F32, kind="ExternalInput")
    idxrow_d = nc.dram_tensor("idxrow", [nsup, SW], F32, kind="ExternalInput")
    idxcol_d = nc.dram_tensor("idxcol", [nsup, NB, SUPER], F32, kind="ExternalInput")
    nodeTaug_d = nc.dram_tensor("nodeTaug", [D + 1, bpc * NB], F32, kind="ExternalInput")
    nodeRg_d = nc.dram_tensor("nodeRg", [ngrp, NB, 8 * D], F32, kind="ExternalInput")
    wrhs_d = nc.dram_tensor("wrhs", [D + 1, DO], F32, kind="ExternalInput")
    wet_d = nc.dram_tensor("wet", [D, DO], F32, kind="ExternalInput")
    iotac_d = nc.dram_tensor("iotac", [NB, 1], F32, kind="ExternalInput")
    iotar_d = nc.dram_tensor("iotar", [NB, NB], F32, kind="ExternalInput")
    if not ln1_triv:
        g1_d = nc.dram_tensor("g1", [NB, DO], F32, kind="ExternalInput")
        b1_d = nc.dram_tensor("b1", [NB, DO], F32, kind="ExternalInput")
    if not ln2_triv:
        g2_d = nc.dram_tensor("g2", [NB, D], F32, kind="ExternalInput")
        b2_d = nc.dram_tensor("b2", [NB, D], F32, kind="ExternalInput")
    out_d = nc.dram_tensor("out", [ngrp, NB, 8 * D], F32, kind="ExternalOutput")

    with TileContext(nc) as tc:
        with (
            tc.tile_pool(name="const", bufs=1) as const,
            tc.tile_pool(name="work", bufs=3) as work,
            tc.tile_pool(name="stat", bufs=3) as stat,
            tc.tile_pool(name="py", bufs=2, space="PSUM") as psum_y,
            tc.tile_pool(name="pagg", bufs=2, space="PSUM") as psum_agg,
            tc.tile_pool(name="pnp", bufs=2, space="PSUM") as psum_np,
        ):
            # ---- persistent tiles ----
            wrhs_t = const.tile([D + 1, DO], F32)
            nc.sync.dma_start(out=wrhs_t[:], in_=wrhs_d[:])
            wet_t = const.tile([D, DO], F32)
            nc.sync.dma_start(out=wet_t[:], in_=wet_d[:])
            iotac_t = const.tile([NB, 1], F32)
            nc.sync.dma_start(out=iotac_t[:], in_=iotac_d[:])
            iotar_t = const.tile([NB, NB], F32)
            nc.sync.dma_start(out=iotar_t[:], in_=iotar_d[:])
            eps_t = const.tile([NB, 1], F32)
            nc.gpsimd.memset(eps_t[:], EPS)
            node_proj = const.tile([NB, NPCOL], F32)
            agg_sb = const.tile([NB, bpc * D], F32)
            if not ln1_triv:
                g1_t = const.tile([NB, DO], F32)
                nc.sync.dma_start(out=g1_t[:], in_=g1_d[:])
                b1_t = const.tile([NB, DO], F32)
                nc.sync.dma_start(out=b1_t[:], in_=b1_d[:])
            if not ln2_triv:
                g2_t = const.tile([NB, D], F32)
                nc.sync.dma_start(out=g2_t[:], in_=g2_d[:])
                b2_t = const.tile([NB, D], F32)
                nc.sync.dma_start(out=b2_t[:], in_=b2_d[:])

            # ---- phase A: node_proj[b] = node_blk @ Wn.T + b (bias via
            # ones-row augmentation of the stationary operand) ----
            for b in range(bpc):
                nt = work.tile([D + 1, NB], F32, tag="nt")
                nc.sync.dma_start(out=nt[:], in_=nodeTaug_d[:, b * NB:(b + 1) * NB])
                npp = psum_np.tile([NB, DO], F32, tag="npp")
                nc.tensor.matmul(npp[:], lhsT=nt[:], rhs=wrhs_t[:], start=True, stop=True)
                nc.scalar.copy(node_proj[:, b * DO:(b + 1) * DO], npp[:])

            # ---- phase B: edge supertiles ----
            aggp = None
            for u in range(nsup):
                et = work.tile([D, SW], F32, tag="et")
                nc.sync.dma_start(out=et[:], in_=edgeT_d[u])
                ir = work.tile([1, SW], F32, tag="ir")
                nc.sync.dma_start(out=ir[:], in_=idxrow_d[u:u + 1, :])
                ic = work.tile([NB, SUPER], F32, tag="ic")
                nc.sync.dma_start(out=ic[:], in_=idxcol_d[u])

                # one-hot transpose: O_T[j, e] = (idx[e] == j)   [128n x SW]
                ib = work.tile([NB, SW], F32, tag="ib")
                nc.gpsimd.partition_broadcast(ib[:], ir[:1, :])
                ot = work.tile([NB, SW], F32, tag="ot")
                nc.vector.tensor_tensor(
                    out=ot[:], in0=iotac_t[:].to_broadcast([NB, SW]), in1=ib[:],
                    op=OP.is_equal)
                # one-hot: O[e, j] = (idx[e] == j)  viewed [128e x (SUPER x 128n)]
                o_t = work.tile([NB, SW], F32, tag="o_t")
                nc.vector.tensor_tensor(
                    out=o_t[:].rearrange("p (s x) -> p s x", s=SUPER),
                    in0=ic[:].rearrange("p (s o) -> p s o", o=1).to_broadcast([NB, SUPER, NB]),
                    in1=iotar_t[:].rearrange("(p o) x -> p o x", o=1).to_broadcast([NB, SUPER, NB]),
                    op=OP.is_equal)

                # y[e, o] = node_proj[blk] expanded by O_T  +  edge @ We.T
                y = psum_y.tile([NB, SW], F32, tag="y")
                for t in range(SUPER):
                    blk = bs[u * SUPER + t]
                    ysl = y[:, t * NB:(t + 1) * NB]
                    nc.tensor.matmul(
                        ysl, lhsT=ot[:, t * NB:(t + 1) * NB],
                        rhs=node_proj[:, blk * DO:(blk + 1) * DO],
                        start=True, stop=False)
                    nc.tensor.matmul(
                        ysl, lhsT=et[:, t * NB:(t + 1) * NB], rhs=wet_t[:],
                        start=False, stop=True)

                y3 = y[:].rearrange("p (s x) -> p s x", s=SUPER)
                mu = stat.tile([NB, SUPER], F32, tag="mu")
                nc.vector.reduce_sum(out=mu[:], in_=y3, axis=AX.X)
                ysq = work.tile([NB, SW], F32, tag="ysq")
                nc.scalar.activation(ysq[:], y[:], AF.Square)
                ss = stat.tile([NB, SUPER], F32, tag="ss")
                nc.vector.reduce_sum(
                    out=ss[:], in_=ysq[:].rearrange("p (s x) -> p s x", s=SUPER),
                    axis=AX.X)
                m2 = stat.tile([NB, SUPER], F32, tag="m2")
                nc.vector.tensor_tensor(out=m2[:], in0=mu[:], in1=mu[:], op=OP.mult)
                vr = stat.tile([NB, SUPER], F32, tag="vr")
                nc.vector.tensor_scalar(
                    out=m2[:], in0=m2[:], scalar1=1.0 / DO, scalar2=None, op0=OP.mult)
                nc.vector.tensor_tensor(out=vr[:], in0=ss[:], in1=m2[:], op=OP.subtract)
                std = stat.tile([NB, SUPER], F32, tag="std")
                nc.scalar.activation(std[:], vr[:], AF.Sqrt, bias=eps_t[:], scale=1.0 / DO)
                rs = stat.tile([NB, SUPER], F32, tag="rs")
                nc.vector.reciprocal(rs[:], std[:])
                nmr = stat.tile([NB, SUPER], F32, tag="nmr")
                nc.vector.tensor_tensor(out=nmr[:], in0=mu[:], in1=rs[:], op=OP.mult)
                nc.vector.tensor_scalar(
                    out=nmr[:], in0=nmr[:], scalar1=-1.0 / DO, scalar2=None, op0=OP.mult)

                sig = work.tile([NB, SUPER * D], F32, tag="sig")
                tah = work.tile([NB, SUPER * D], F32, tag="tah")
                if ln1_triv:
                    # filter half: sigmoid((y - mean) * rs) fused on ACT
                    for t in range(SUPER):
                        nc.scalar.activation(
                            sig[:, t * D:(t + 1) * D], y[:, t * NB:t * NB + D],
                            AF.Sigmoid, bias=nmr[:, t:t + 1], scale=rs[:, t:t + 1])
                    # core half: normalize on DVE, then one packed tanh
                    uc = work.tile([NB, SUPER * D], F32, tag="uc")
                    for t in range(SUPER):
                        nc.vector.tensor_scalar(
                            out=uc[:, t * D:(t + 1) * D],
                            in0=y[:, t * NB + D:(t + 1) * NB],
                            scalar1=rs[:, t:t + 1], scalar2=nmr[:, t:t + 1],
                            op0=OP.mult, op1=OP.add)
                    nc.scalar.activation(tah[:], uc[:], AF.Tanh)
                else:
                    un = work.tile([NB, SW], F32, tag="un")
                    for t in range(SUPER):
                        nc.vector.tensor_scalar(
                            out=un[:, t * NB:(t + 1) * NB],
                            in0=y[:, t * NB:(t + 1) * NB],
                            scalar1=rs[:, t:t + 1], scalar2=nmr[:, t:t + 1],
                            op0=OP.mult, op1=OP.add)
                    un3 = un[:].rearrange("p (s x) -> p s x", s=SUPER)
                    g1b = g1_t[:].rearrange("(p o) x -> p o x", o=1).to_broadcast([NB, SUPER, DO])
                    b1b = b1_t[:].rearrange("(p o) x -> p o x", o=1).to_broadcast([NB, SUPER, DO])
                    nc.vector.tensor_tensor(out=un3, in0=un3, in1=g1b, op=OP.mult)
                    nc.gpsimd.tensor_tensor(out=un3, in0=un3, in1=b1b, op=OP.add)
                    nc.scalar.activation(
                        sig[:].rearrange("p (s x) -> p s x", s=SUPER),
                        un3[:, :, 0:D], AF.Sigmoid)
                    nc.scalar.activation(
                        tah[:].rearrange("p (s x) -> p s x", s=SUPER),
                        un3[:, :, D:DO], AF.Tanh)

                msg = work.tile([NB, SUPER * D], F32, tag="msg")
                nc.vector.tensor_tensor(out=msg[:], in0=sig[:], in1=tah[:], op=OP.mult)

                # segment-sum into per-block PSUM accumulators
                for t in range(SUPER):
                    s = u * SUPER + t
                    blk = bs[s]
                    first = s == 0 or bs[s - 1] != blk
                    last = s == nsup * SUPER - 1 or bs[s + 1] != blk
                    if first:
                        aggp = psum_agg.tile([NB, D], F32, tag="aggp")
                    nc.tensor.matmul(
                        aggp[:], lhsT=o_t[:, t * NB:(t + 1) * NB],
                        rhs=msg[:, t * D:(t + 1) * D], start=first, stop=last)
                    if last:
                        nc.scalar.copy(agg_sb[:, blk * D:(blk + 1) * D], aggp[:])

            # ---- phase C: LayerNorm(agg) + residual + tanh ----
            for g in range(ngrp):
                nbk = min(8, bpc - g * 8)
                fd = nbk * D
                agg_g = agg_sb[:, g * 8 * D: g * 8 * D + fd]
                a3 = agg_g.rearrange("p (s x) -> p s x", s=nbk)
                nr = work.tile([NB, 8 * D], F32, tag="nr")
                nc.sync.dma_start(out=nr[:, :fd], in_=nodeRg_d[g, :, :fd])
                mu = stat.tile([NB, 8], F32, tag="cmu")
                nc.vector.reduce_sum(out=mu[:, :nbk], in_=a3, axis=AX.X)
                asq = work.tile([NB, 8 * D], F32, tag="asq")
                nc.scalar.activation(asq[:, :fd], agg_g, AF.Square)
                ss = stat.tile([NB, 8], F32, tag="css")
                nc.vector.reduce_sum(
                    out=ss[:, :nbk],
                    in_=asq[:, :fd].rearrange("p (s x) -> p s x", s=nbk), axis=AX.X)
                m2 = stat.tile([NB, 8], F32, tag="cm2")
                nc.vector.tensor_tensor(
                    out=m2[:, :nbk], in0=mu[:, :nbk], in1=mu[:, :nbk], op=OP.mult)
                nc.vector.tensor_scalar(
                    out=m2[:, :nbk], in0=m2[:, :nbk], scalar1=1.0 / D, scalar2=None,
                    op0=OP.mult)
                vr = stat.tile([NB, 8], F32, tag="cvr")
                nc.vector.tensor_tensor(
                    out=vr[:, :nbk], in0=ss[:, :nbk], in1=m2[:, :nbk], op=OP.subtract)
                std = stat.tile([NB, 8], F32, tag="cstd")
                nc.scalar.activation(std[:, :nbk], vr[:, :nbk], AF.Sqrt,
                                     bias=eps_t[:], scale=1.0 / D)
                rs = stat.tile([NB, 8], F32, tag="crs")
                nc.vector.reciprocal(rs[:, :nbk], std[:, :nbk])
                nmr = stat.tile([NB, 8], F32, tag="cnmr")
                nc.vector.tensor_tensor(
                    out=nmr[:, :nbk], in0=mu[:, :nbk], in1=rs[:, :nbk], op=OP.mult)
                nc.vector.tensor_scalar(
                    out=nmr[:, :nbk], in0=nmr[:, :nbk], scalar1=-1.0 / D, scalar2=None,
                    op0=OP.mult)
                un = work.tile([NB, 8 * D], F32, tag="cun")
                for j in range(nbk):
                    nc.vector.tensor_scalar(
                        out=un[:, j * D:(j + 1) * D],
                        in0=agg_g[:, j * D:(j + 1) * D],
                        scalar1=rs[:, j:j + 1], scalar2=nmr[:, j:j + 1],
                        op0=OP.mult, op1=OP.add)
                if not ln2_triv:
                    un3 = un[:, :fd].rearrange("p (s x) -> p s x", s=nbk)
                    g2b = g2_t[:].rearrange("(p o) x -> p o x", o=1).to_broadcast([NB, nbk, D])
                    b2b = b2_t[:].rearrange("(p o) x -> p o x", o=1).to_broadcast([NB, nbk, D])
                    nc.vector.tensor_tensor(out=un3, in0=un3, in1=g2b, op=OP.mult)
                    nc.vector.tensor_tensor(out=un3, in0=un3, in1=b2b, op=OP.add)
                wv = work.tile([NB, 8 * D], F32, tag="cw")
                nc.vector.tensor_tensor(
                    out=wv[:, :fd], in0=un[:, :fd], in1=nr[:, :fd], op=OP.add)
                ov = work.tile([NB, 8 * D], F32, tag="cov")
                nc.scalar.activation(ov[:, :fd], wv[:, :fd], AF.Tanh)
                nc.sync.dma_start(out=out_d[g, :, :fd], in_=ov[:, :fd])

    nc.compile()
    _NC_CACHE[key] = nc
    return nc


def kernel(node_embedding, edge_embedding, W, b, gamma1, beta1, gamma2, beta2, i,
           **_unused):
    node_embedding = np.ascontiguousarray(np.asarray(node_embedding, dtype=np.float32))
    edge_embedding = np.ascontiguousarray(np.asarray(edge_embedding, dtype=np.float32))
    W = np.asarray(W, dtype=np.float32)
    b = np.asarray(b, dtype=np.float32)
    gamma1 = np.asarray(gamma1, dtype=np.float32)
    beta1 = np.asarray(beta1, dtype=np.float32)
    gamma2 = np.asarray(gamma2, dtype=np.float32)
    beta2 = np.asarray(beta2, dtype=np.float32)
    idx = np.asarray(i).astype(np.int64)

    N, d = node_embedding.shape
    E = idx.shape[0]
    assert d == D

    nblk_raw = -(-N // NB)
    bpc = -(-nblk_raw // NCORES)          # blocks per core
    nblk = bpc * NCORES
    npad = nblk * NB

    # ---- sort edges by destination node, bucket into node blocks ----
    order = np.argsort(idx, kind="stable")
    sidx = idx[order]
    sblk = (sidx // NB).astype(np.int64)
    counts = np.bincount(sblk, minlength=nblk)
    cum = np.zeros(nblk + 1, dtype=np.int64)
    np.cumsum(counts, out=cum[1:])
    eblk = int(max(1, -(-counts.max() // NB))) * NB     # padded edges per block
    nsub = eblk // NB                                   # subtiles per block
    S = bpc * nsub
    nsup = -(-S // SUPER)
    spad = nsup * SUPER

    ln1_triv = bool(np.all(gamma1 == 1.0) and np.all(beta1 == 0.0))
    ln2_triv = bool(np.all(gamma2 == 1.0) and np.all(beta2 == 0.0))

    # slot assignment: edge e (sorted) -> core, subtile, lane
    ranks = np.arange(E, dtype=np.int64) - cum[sblk]
    core = sblk // bpc
    lblk = sblk - core * bpc
    sub = lblk * nsub + ranks // NB
    lane = ranks % NB
    flat = (core * (spad * NB) + sub * NB + lane)

    # gather edge features / local idx into padded slot arrays
    gidx = np.full(NCORES * spad * NB, -1, dtype=np.int64)
    gidx[flat] = order
    lidx = np.full(NCORES * spad * NB, -1.0, dtype=np.float32)
    lidx[flat] = (sidx - sblk * NB).astype(np.float32)

    efeat = np.zeros((NCORES * spad * NB, D), dtype=np.float32)
    valid = gidx >= 0
    efeat[valid] = edge_embedding[gidx[valid]]

    node_pad = np.zeros((npad, D), dtype=np.float32)
    node_pad[:N] = node_embedding

    ngrp = (bpc + 7) // 8
    gpad = ngrp * 8 * NB                                # padded rows per core (phase C)

    in_maps = []
    iotac = np.arange(NB, dtype=np.float32).reshape(NB, 1)
    iotar = np.tile(np.arange(NB, dtype=np.float32), (NB, 1))
    wrhs = np.concatenate([W[:, :D].T, b.reshape(1, DO)], axis=0).astype(np.float32)
    wet = np.ascontiguousarray(W[:, D:].T)
    for c in range(NCORES):
        ef = efeat[c * spad * NB:(c + 1) * spad * NB]
        li = lidx[c * spad * NB:(c + 1) * spad * NB]
        edgeT = np.ascontiguousarray(
            ef.reshape(nsup, SUPER * NB, D).transpose(0, 2, 1))
        idxrow = np.ascontiguousarray(li.reshape(nsup, SUPER * NB))
        idxcol = np.ascontiguousarray(
            li.reshape(nsup, SUPER, NB).transpose(0, 2, 1))
        nslice = node_pad[c * bpc * NB:(c + 1) * bpc * NB]
        nodeTaug = np.concatenate(
            [nslice.T, np.ones((1, bpc * NB), np.float32)], axis=0)
        nR = np.zeros((gpad, D), dtype=np.float32)
        nR[:bpc * NB] = nslice
        nodeRg = np.ascontiguousarray(
            nR.reshape(ngrp, 8, NB, D).transpose(0, 2, 1, 3).reshape(ngrp, NB, 8 * D))
        m = {
            "edgeT": edgeT, "idxrow": idxrow, "idxcol": idxcol,
            "nodeTaug": np.ascontiguousarray(nodeTaug), "nodeRg": nodeRg,
            "wrhs": wrhs, "wet": wet, "iotac": iotac, "iotar": iotar,
        }
        if not ln1_triv:
            m["g1"] = np.tile(gamma1.reshape(1, DO), (NB, 1))
            m["b1"] = np.tile(beta1.reshape(1, DO), (NB, 1))
        if not ln2_triv:
            m["g2"] = np.tile(gamma2.reshape(1, D), (NB, 1))
            m["b2"] = np.tile(beta2.reshape(1, D), (NB, 1))
        in_maps.append(m)

    nc = _build_nc(nsub, nsup, bpc, ln1_triv, ln2_triv)

    from concourse.bass_utils import run_bass_kernel_spmd
    trace = bool(int(os.environ.get("KERNEL_TRACE", "0")))
    res = run_bass_kernel_spmd(nc, in_maps, list(range(NCORES)), trace=trace)
    kernel._last_exec_ns = res.exec_time_ns

    outs = []
    for c in range(NCORES):
        o = res.results[c]["out"]                       # [ngrp, NB, 8*D]
        o = o.reshape(ngrp, NB, 8, D).transpose(0, 2, 1, 3).reshape(gpad, D)
        outs.append(o[:bpc * NB])
    full = np.concatenate(outs, axis=0)[:N]
    return full.astype(np.float32)
